# revision 23
# baseline (speedup 1.0000x reference)
"""Block-sparse linear layer (x @ (mask*W).T + bias) on 8 TRN2 NeuronCores.

Strategy: data-parallel over batch rows; each core computes 1024 rows.
Mixed-precision compute: per output block-row, the latin-square mask keeps
8 of 16 k-blocks (256 cols each).  Six of those blocks run as bf16 matmuls
(K=128 subtiles, N=512) and the two blocks that fall in S={0,4,8,12} run as
fp8-e4m3 DoubleRow matmuls (K=256 per pass - double-pumped, 2x PE rate).
W is pre-scaled by 64 (exact in bf16, keeps the fp8 operand out of the
denormal range) and the eviction fuses out = psum/64 + bias.
Offline-exact error for this split: 1.61e-2 absmax-rel (gate 2e-2).

Schedule notes:
- all W resident in SBUF (gpsimd queue); fp8 W for the first 4 o-tiles in
  one merged DMA so the fp8 phase starts on a single semaphore.
- x on the sync queue, fp8 x first (small, unblocks real fp8 matmuls that
  cover the startup window after the warmup matmuls).
- evictions split per tile across vector (h0, out via sync) and scalar
  (h1, out via scalar queue) so psum frees ~750ns after the stop matmul.
- fp8 regions alternate first/last in consecutive pairs so they stay
  contiguous across pair boundaries (the first fp8 matmul after a bf16
  run pays a ~190ns mode-switch penalty).
- final o-tile is emitted half-major into two separate psum tiles so its
  first-half eviction overlaps the second half's accumulation.
"""

import sys
import types

import numpy as np
import ml_dtypes

BATCH = 8192
SIZE = 4096
NB = 16
BLOCK = 256
NCORES = 8
MC = BATCH // NCORES  # 1024 rows per core
P = 128
OT = SIZE // P  # 32 o-tiles
MM_N = 512
SC = 64.0  # weight pre-scale (power of two, exact in bf16)
S_FP8 = (0, 4, 8, 12)  # blocks computed in fp8 (latin-square plan)

_BUILD_CACHE = {}


def _install_ntff_hook():
    if "antenv.axon_hooks" in sys.modules:
        return
    try:
        from trn_agent_boot.trn_boot import _ntff_profile_via_ctypes

        hook = _ntff_profile_via_ctypes("/opt/axon/libaxon_pjrt.so")
        mod = types.ModuleType("antenv.axon_hooks")
        mod.get_axon_ntff_profile_hook = lambda: hook
        sys.modules["antenv.axon_hooks"] = mod
    except Exception:
        pass


def _block_keep_from_mask(mask):
    m4 = np.asarray(mask).reshape(NB, BLOCK, NB, BLOCK)
    keep = m4[:, 0, :, 0]
    uniform = np.all(m4 == keep[:, None, :, None])
    return keep if uniform else None


def _make_plan(keep):
    latin = np.array([[((i + j) % 16) >= 8 for j in range(NB)] for i in range(NB)])
    use_fp8 = bool(np.array_equal(keep, latin))

    F, B = [], []
    for i in range(NB):
        kept = [j for j in range(NB) if keep[i, j]]
        if use_fp8:
            F.append([j for j in kept if j in S_FP8])
            B.append([j for j in kept if j not in S_FP8])
        else:
            F.append([])
            B.append(kept)

    # First group: tiles (0,1,30,31) - block-rows 0 and 15 have identical
    # bf16 block sets under the latin mask, so every startup x slab feeds
    # all four tiles (minimum startup x footprint).  Falls back to (0,1,2,3)
    # when the rows differ.
    if use_fp8 and B[0] == B[NB - 1]:
        groups = [(0, 1, 2 * NB - 2, 2 * NB - 1)] + [
            (2 * i, 2 * i + 1) for i in range(1, NB - 1)
        ]
    else:
        groups = [(0, 1, 2, 3)] + [(2 * i, 2 * i + 1) for i in range(2, NB)]

    xord, ford = [], []
    for ts in groups:
        for t in ts:
            for j in F[t // 2]:
                if j not in ford:
                    ford.append(j)
            for j in B[t // 2]:
                if j not in xord:
                    xord.append(j)

    n_xslot = 2 * len(xord)
    n_fslot = len(ford)
    NBT = max(1, max(2 * len(b) for b in B))
    NFT = max(len(f) for f in F)

    tiles_xs, tiles_fs = [], []
    for t in range(OT):
        i = t // 2
        xs = []
        for j in B[i]:
            g = xord.index(j)
            xs.extend((2 * g, 2 * g + 1))
        if not xs and not F[i]:
            xs = [0]
        tiles_xs.append(tuple(xs))
        tiles_fs.append(tuple(ford.index(j) for j in F[i]))

    w_bytes_per_part = OT * (NBT * P * 2 + NFT * 2 * P)
    resident = w_bytes_per_part <= 120 * 1024

    plan = (
        n_xslot,
        n_fslot,
        tuple(tiles_fs),
        tuple(tiles_xs),
        NBT,
        NFT,
        resident,
        tuple(groups),
    )
    return plan, xord, ford


def _build(plan):
    import concourse.mybir as mybir
    import concourse.tile as tile
    from concourse import bacc

    (n_xslot, n_fslot, tiles_fs, tiles_xs, NBT, NFT, resident, groups) = plan
    wpos = {t: i for i, t in enumerate(t for g in groups for t in g)}

    bf16, f32, f8 = mybir.dt.bfloat16, mybir.dt.float32, mybir.dt.float8e4
    DR = mybir.MatmulPerfMode.DoubleRow
    IDENT = mybir.ActivationFunctionType.Identity
    MUL, ADD = mybir.AluOpType.mult, mybir.AluOpType.add
    INV = 1.0 / SC

    nc = bacc.Bacc("TRN2", target_bir_lowering=False)
    xtb_d = nc.declare_dram_parameter("xtb", [P, n_xslot, MC], bf16, isOutput=False)
    if n_fslot:
        xf8_d = nc.declare_dram_parameter("xf8", [P, n_fslot, 2, MC], f8, isOutput=False)
        wf8_d = nc.declare_dram_parameter("wf8", [P, OT, NFT, 2, P], f8, isOutput=False)
    wtb_d = nc.declare_dram_parameter("wtb", [OT, P, NBT, P], bf16, isOutput=False)
    bias_d = nc.declare_dram_parameter("biast", [P, OT], f32, isOutput=False)
    out_d = nc.declare_dram_parameter("out", [OT, P, MC], bf16, isOutput=True)

    W_BUFS = OT if resident else 8

    with tile.TileContext(nc) as tc:
        with (
            tc.tile_pool(name="const", bufs=1) as const_pool,
            tc.tile_pool(name="xpool", bufs=1) as xpool,
            tc.tile_pool(name="wbpool", bufs=W_BUFS) as wbpool,
            tc.tile_pool(name="opool", bufs=4) as opool,
            tc.tile_pool(name="psum", bufs=4, space="PSUM") as psum_pool,
        ):
            bias_tile = const_pool.tile([P, OT], f32)
            nc.scalar.dma_start(out=bias_tile[:], in_=bias_d[:])

            # PE warmup: keep the HAM un-throttle window alive while the
            # first x/W DMAs land.
            warm = const_pool.tile([P, MM_N], bf16, name="warm")
            nc.vector.memset(warm[:], 0)
            warm_ps = psum_pool.tile([P, MM_N], f32, name="warm_ps", tag="ps")
            N_WARM = 12
            for i in range(N_WARM):
                nc.tensor.matmul(
                    warm_ps[:],
                    lhsT=warm[:, 0:P],
                    rhs=warm[:],
                    start=(i == 0),
                    stop=(i == N_WARM - 1),
                )

            # ---- DMA issue ----
            # sync queue: fp8 x slots for the first groups, then bf16 x in
            # consumption order (second fp8 piece deferred - first needed
            # only ~40us in).
            # Startup-critical transfers are interleaved in need order across
            # all three DMA-capable queues (they share the DMA engines, so
            # per-queue FIFO order plus need-ordering across queues keeps the
            # aggregate bandwidth on the critical path).  Everything else is
            # gated behind a dummy read of a mid-startup x tile.
            f_tile = None
            x_tiles = {}
            wtb_tiles = {}
            wf_tile = None

            def x_dma(g, eng):
                xg = xpool.tile([P, 2, MC], bf16, name=f"x_g{g}", uniquify=False)
                eng.dma_start(out=xg[:], in_=xtb_d[:, 2 * g : 2 * g + 2, :])
                x_tiles[g] = xg

            def wtb_dma(t, eng):
                wb = wbpool.tile([P, NBT, P], bf16, name="wtb_tile")
                eng.dma_start(out=wb[:], in_=wtb_d[t])
                wtb_tiles[t] = wb

            if n_fslot:
                f_tile = xpool.tile([P, n_fslot, 2, MC], f8, name="xf8t", uniquify=False)
                wf_tile = xpool.tile([P, OT, NFT, 2, P], f8, name="wf8t", uniquify=False)

            n_xg = n_xslot // 2
            if resident and n_fslot:
                half = 1 + max(cs for t in groups[0] for cs in tiles_fs[t])
                # sync: first group's fp8 W + fp8 x slots (per-slot pieces -
                # the first 8 fp8 matmuls need only slot 0), then bf16 x
                # groups in consumption order (FIFO keeps the tail behind
                # the critical head), late fp8 x piece at the end.
                for c in range(half):
                    nc.sync.dma_start(
                        out=f_tile[:, c : c + 1], in_=xf8_d[:, c : c + 1]
                    )
                for g in range(n_xg):
                    x_dma(g, nc.sync)
                if half < n_fslot:
                    nc.sync.dma_start(
                        out=f_tile[:, half:n_fslot], in_=xf8_d[:, half:n_fslot]
                    )
                # gpsimd: first pair's bf16 W immediately; the second pair's
                # gated on the first x group, the bulk gated on a mid-startup
                # x tile, so W cannot steal HBM bandwidth from the critical
                # x stream (each DMA needs its own WAW predecessor - ready
                # instructions overtake blocked ones regardless of priority).
                emis = [t for g in groups for t in g]
                nc.gpsimd.dma_start(out=wf_tile[:, 0:4], in_=wf8_d[:, 0:4])
                for t in groups[0]:
                    wtb_dma(t, nc.gpsimd)
                gate_g = min(5, n_xg - 1)
                gate_src = x_tiles[gate_g][0:1, 0:1, 0:1]

                def gated_wtb_dma(t, src):
                    wb = wbpool.tile([P, NBT, P], bf16, name="wtb_tile")
                    nc.gpsimd.tensor_scalar_add(wb[0:1, 0:1, 0:1], src, 0.0)
                    nc.gpsimd.dma_start(out=wb[:], in_=wtb_d[t])
                    wtb_tiles[t] = wb

                gated_wtb_dma(emis[4], gate_src)
                gated_wtb_dma(emis[5], gate_src)
                nc.gpsimd.tensor_scalar_add(
                    wf_tile[0:1, 4:5, 0:1, 0:1, 0:1], gate_src, 0.0
                )
                nc.gpsimd.dma_start(out=wf_tile[:, 4:OT], in_=wf8_d[:, 4:OT])
                for t in emis[6:]:
                    gated_wtb_dma(t, gate_src)
            else:
                if n_fslot:
                    nc.scalar.dma_start(out=f_tile[:], in_=xf8_d[:])
                    nc.gpsimd.dma_start(out=wf_tile[:], in_=wf8_d[:])
                for g in range(n_xg):
                    x_dma(g, nc.sync)
                for t in range(4):
                    wtb_dma(t, nc.gpsimd)
                if resident:
                    for t in range(4, OT):
                        wtb_dma(t, nc.gpsimd)

            def x_ap(slot):
                return x_tiles[slot // 2][:, slot % 2, :]

            # ---- compute emission ----
            def n_units(t):
                return len(tiles_fs[t]) + len(tiles_xs[t])

            def emit_f(t, fi, ps_t, n_done, h_list=(0, 1), ps_off=0):
                total = n_units(t)
                first = n_done[t] == 0
                n_done[t] += 1
                last = n_done[t] == total
                cs = tiles_fs[t][fi]
                for h in h_list:
                    sl = slice(h * MM_N - ps_off, (h + 1) * MM_N - ps_off)
                    nc.tensor.matmul(
                        ps_t[:, sl],
                        lhsT=wf_tile[:, wpos[t], fi, :, :],
                        rhs=f_tile[:, cs, :, h * MM_N : (h + 1) * MM_N],
                        start=first,
                        stop=last,
                        perf_mode=DR,
                    )

            def emit_x(t, slot, ps_t, n_done, h_list=(0, 1), ps_off=0):
                total = n_units(t)
                first = n_done[t] == 0
                n_done[t] += 1
                last = n_done[t] == total
                u = tiles_xs[t].index(slot)
                for h in h_list:
                    sl = slice(h * MM_N - ps_off, (h + 1) * MM_N - ps_off)
                    nc.tensor.matmul(
                        ps_t[:, sl],
                        lhsT=wtb_tiles[t][:, u, :],
                        rhs=x_ap(slot)[:, h * MM_N : (h + 1) * MM_N],
                        start=first,
                        stop=last,
                    )

            def evict(t, ps_t):
                # h0 on vector (+sync out-queue), h1 on scalar (+scalar
                # out-queue): psum frees ~750ns after the stop matmul.
                o = opool.tile([P, MC], bf16, name="o_tile")
                s0, s1 = slice(0, MM_N), slice(MM_N, 2 * MM_N)
                nc.vector.tensor_scalar(
                    o[:, s0], ps_t[:, s0], INV, bias_tile[:, t : t + 1], MUL, ADD
                )
                nc.sync.dma_start(out=out_d[t, :, s0], in_=o[:, s0])
                nc.scalar.activation(
                    o[:, s1], ps_t[:, s1], IDENT, bias=bias_tile[:, t : t + 1], scale=INV
                )
                nc.sync.dma_start(out=out_d[t, :, s1], in_=o[:, s1])

            def emit_dr(ts, ps, n_done):
                units = [
                    (tiles_fs[t][fi], t, fi)
                    for t in ts
                    for fi in range(len(tiles_fs[t]))
                ]
                for _, t, fi in sorted(units):
                    emit_f(t, fi, ps[t], n_done)

            def emit_bf(ts, ps, n_done, order=None):
                if order is None:
                    order = [
                        (s, t) for s in range(n_xslot) for t in ts if s in tiles_xs[t]
                    ]
                for s, t in order:
                    emit_x(t, s, ps[t], n_done)
                    if n_done[t] == n_units(t):
                        evict(t, ps[t])

            def emit_group(ts, dr_first, ramp=False):
                ps = {t: psum_pool.tile([P, MC], f32, name="ps", tag="ps") for t in ts}
                n_done = {t: 0 for t in ts}
                order = None
                if ramp and len(ts) == 4:
                    # warm ramp for the first group: the first three x blocks
                    # run tile-major on the first two tiles (so only their W
                    # tiles gate the start), everyone else catches up after.
                    a, b = ts[:2], ts[2:]
                    slots_a = [s for s in range(n_xslot) if s in tiles_xs[a[0]]]
                    lead = slots_a[:6]
                    order = [(s, t) for t in a for s in lead if s in tiles_xs[t]]
                    order += [
                        (s, t)
                        for s in range(n_xslot)
                        if s not in lead
                        for t in ts
                        if s in tiles_xs[t]
                    ]
                    order += [
                        (s, t) for t in b for s in lead if s in tiles_xs[t]
                    ]
                if dr_first:
                    emit_dr(ts, ps, n_done)
                    emit_bf(ts, ps, n_done, order)
                else:
                    emit_bf(ts, ps, n_done, order)
                    emit_dr(ts, ps, n_done)
                    for t in ts:
                        evict(t, ps[t])

            def emit_tail(ts, dr_first):
                """Tile-major final group; last tile half-major into two
                separate psum tiles so its h0 eviction overlaps h1."""
                first_tiles = ts[:-1]
                ps = {
                    t: psum_pool.tile([P, MC], f32, name="ps", tag="ps")
                    for t in first_tiles
                }
                n_done = {t: 0 for t in ts}
                for t in first_tiles:
                    fl = list(range(len(tiles_fs[t])))
                    if dr_first:
                        for fi in fl:
                            emit_f(t, fi, ps[t], n_done)
                    for s in tiles_xs[t]:
                        emit_x(t, s, ps[t], n_done)
                    if not dr_first:
                        for fi in fl:
                            emit_f(t, fi, ps[t], n_done)
                    evict(t, ps[t])
                t = ts[-1]
                pa = psum_pool.tile([P, MM_N], f32, name="ps", tag="ps")
                pb = psum_pool.tile([P, MM_N], f32, name="ps", tag="ps")
                o = opool.tile([P, MC], bf16, name="o_tile")
                q = MM_N // 2
                for h, ps_t in ((0, pa), (1, pb)):
                    n_done[t] = 0
                    # keep the fp8 run contiguous with the previous region
                    for fi in range(len(tiles_fs[t])):
                        emit_f(t, fi, ps_t, n_done, h_list=(h,), ps_off=h * MM_N)
                    for s in tiles_xs[t]:
                        emit_x(t, s, ps_t, n_done, h_list=(h,), ps_off=h * MM_N)
                    # evict this half split across both engines
                    lo = h * MM_N
                    sv = slice(lo, lo + q)
                    ss = slice(lo + q, lo + 2 * q)
                    pv = slice(0, q)
                    psl = slice(q, 2 * q)
                    nc.vector.tensor_scalar(
                        o[:, sv], ps_t[:, pv], INV, bias_tile[:, t : t + 1], MUL, ADD
                    )
                    nc.sync.dma_start(out=out_d[t, :, sv], in_=o[:, sv])
                    nc.scalar.activation(
                        o[:, ss], ps_t[:, psl], IDENT,
                        bias=bias_tile[:, t : t + 1], scale=INV,
                    )
                    (nc.scalar if h == 1 else nc.sync).dma_start(
                        out=out_d[t, :, ss], in_=o[:, ss]
                    )

            for gi, ts in enumerate(groups):
                if not resident:
                    for t in ts:
                        if t not in wtb_tiles:
                            wtb_dma(t)
                dr_first = (gi % 2) == 1
                if gi == len(groups) - 1 and len(ts) >= 2:
                    emit_tail(ts, dr_first)
                elif gi == 0:
                    emit_group(ts, dr_first=True, ramp=True)
                else:
                    emit_group(ts, dr_first)
    nc.compile()
    return nc


def _get_kernel(plan):
    if plan not in _BUILD_CACHE:
        _BUILD_CACHE[plan] = _build(plan)
    return _BUILD_CACHE[plan]


def kernel(x, weight, bias, mask, _trace=False):
    import concourse.mybir as mybir
    from concourse.bass_utils import run_bass_kernel_spmd

    _install_ntff_hook()

    f8_np = mybir.dt.np(mybir.dt.float8e4)
    bf16_np = ml_dtypes.bfloat16

    x = np.asarray(x)
    weight = np.asarray(weight)
    bias = np.asarray(bias, dtype=np.float32)
    keep = _block_keep_from_mask(mask)
    if keep is None:
        weight = np.where(np.asarray(mask), weight, 0.0).astype(np.float32)
        keep = np.ones((NB, NB), dtype=bool)
    plan, xord, ford = _make_plan(keep)
    (n_xslot, n_fslot, tiles_fs, tiles_xs, NBT, NFT, _, _) = plan

    nc = _get_kernel(plan)

    ws = (weight * SC).astype(np.float32)
    w4 = ws.reshape(OT, P, SIZE)  # [t, q, k]

    wtb = np.zeros((OT, P, NBT, P), dtype=bf16_np)
    for t in range(OT):
        if not keep[t // 2].any():
            continue
        xs = tiles_xs[t]
        subs = [2 * xord[s // 2] + (s % 2) for s in xs]
        sel = w4[t].reshape(P, SIZE // P, P)[:, subs, :]  # [q, u, p]
        wtb[t][:, : len(xs), :] = sel.transpose(2, 1, 0).astype(bf16_np)

    wf8 = None
    groups = plan[7]
    wpos = {t: i for i, t in enumerate(t for g in groups for t in g)}
    if n_fslot:
        wf8 = np.zeros((P, OT, NFT, 2, P), dtype=f8_np)
        for t in range(OT):
            for f, cs in enumerate(tiles_fs[t]):
                j = ford[cs]
                blk = w4[t][:, j * BLOCK : (j + 1) * BLOCK]  # [q, 256]
                blk = blk.reshape(P, 2, P)  # [q, e, p]
                wf8[:, wpos[t], f, :, :] = blk.transpose(2, 1, 0).astype(f8_np)

    biast = np.ascontiguousarray(bias.reshape(OT, P).T, dtype=np.float32)

    xsubs = []
    for g in range(n_xslot // 2):
        xsubs.extend((2 * xord[g], 2 * xord[g] + 1))

    in_maps = []
    for c in range(NCORES):
        xc = x[c * MC : (c + 1) * MC, :]  # [MC, SIZE]
        x3 = xc.reshape(MC, SIZE // P, P)  # [m, sub, p]
        xtb = np.ascontiguousarray(
            x3[:, xsubs, :].transpose(2, 1, 0)
        ).astype(bf16_np)  # [P, n_xslot, MC]
        im = {"xtb": xtb, "wtb": wtb, "biast": biast}
        if n_fslot:
            xf = np.empty((P, n_fslot, 2, MC), dtype=f8_np)
            for cslot, j in enumerate(ford):
                blk = x3[:, 2 * j : 2 * j + 2, :].astype(f8_np)  # [m, e, p]
                xf[:, cslot] = blk.transpose(2, 1, 0)
            im["xf8"] = xf
            im["wf8"] = wf8
        in_maps.append(im)

    res = run_bass_kernel_spmd(nc, in_maps, list(range(NCORES)), trace=_trace)

    out = np.empty((BATCH, SIZE), dtype=np.float32)
    for c in range(NCORES):
        o = np.asarray(res.results[c]["out"])  # [OT, P, MC] bf16
        out[c * MC : (c + 1) * MC, :] = o.reshape(SIZE, MC).T.astype(np.float32)
    if _trace:
        return out, res
    return out
